# revision 1
# baseline (speedup 1.0000x reference)
"""Two-layer GAT (PyG GATConv semantics) on 8 Trainium2 NeuronCores via Bass.

Strategy (per sharding hint): nodes sharded across 8 cores (3750 each).
 - Phase B: each core computes z = [h | alpha_src | alpha_dst] for its nodes
   as x @ [W1 | W1@A_src | W1@A_dst] (alpha reduction folded into the matmul
   by host-precomputing W1 @ A).
 - Phase C: AllGather z -> full z table on every core.
 - Phase D: edges partitioned by destination, sorted by dst, grouped into
   128-edge chunks within 128-node dst blocks. Per chunk: indirect-DMA gather
   of z[src] rows, one-hot selection matrix S (built on device from the
   dst-slot metadata), attention logits exp(leakyrelu(asrc+adst)) (no max
   subtraction needed: logits are small, softmax is shift-invariant),
   aggregation + softmax denominator via S^T @ msg matmuls accumulated in
   PSUM per dst block. Then normalize, +b1, ELU -> h2; z2 = h2 @ [W2|W2@A2].
 - Phase E: AllGather z2 (tiny).
 - Phase F: same edge pass for layer 2 (1 head, 6 cols), +b2 -> output.

kernel(**inputs) takes FULL inputs, shards internally, returns [30000, 6].
"""
import os
import sys

sys.path.insert(0, "/opt/trn_rl_repo")

import numpy as np

import concourse.bass as bass
import concourse.mybir as mybir
import concourse.tile as tile
from concourse.vector_clock import ScopedClock

P = 128
F32 = mybir.dt.float32
I32 = mybir.dt.int32
AF = mybir.ActivationFunctionType
ALU = mybir.AluOpType
NEG_SLOPE = 0.2

# ---------------------------------------------------------------------------
# Walrus workarounds: this environment's walrus build cannot encode semaphore
# waits on Drain instructions (and at most one on any other instruction).
# ---------------------------------------------------------------------------


def _patched_drain_and_barrier(self, tick_clock, wait_clock):
    nc = self.nc
    probe = nc.sync.nop(nofuse=True).ins
    wait_clock.add_sem_waits(probe, ScopedClock({None: tick_clock.global_clock}))
    waits = list(probe.sync_info.on_wait or []) if probe.sync_info else []
    if probe.sync_info is not None:
        probe.sync_info.on_wait = []
    by_num = {h.num: h for h in self.sems.allocated().values()}
    for w in waits:
        h = by_num.get(w.id)
        assert h is not None, f"no semaphore handle for {w.id} {w.ant_name}"
        nc.sync.wait_ge(h, w.wait_value)
    nc.sync.drain()
    nc.all_engine_barrier()
    popped = nc._tile_sem_poison_stack.pop()
    assert popped is self._sem_poison
    nc.clear_and_free_semaphores(list(self.sems.allocated().values()))
    nc.all_engine_barrier()


def _install_tile_patch():
    if getattr(tile.TileContext, "_drain_patch_installed", False):
        return
    tile.TileContext._drain_and_barrier = _patched_drain_and_barrier
    tile.TileContext._drain_patch_installed = True


_install_tile_patch()

_WAIT_LIMIT_ZERO = ("InstDrain", "InstNoOp")


def split_sync_waits(nc, default_limit=1):
    """Move excess semaphore waits onto EventSemaphore insts (same engine)."""
    n_split = 0
    for fn in nc.m.functions:
        for blk in fn.blocks:
            changed = False
            new = []
            for ins in blk.instructions:
                si = ins.sync_info
                waits = list(si.on_wait or []) if si and si.on_wait else []
                limit = 0 if type(ins).__name__ in _WAIT_LIMIT_ZERO else default_limit
                if len(waits) > limit:
                    extra = waits[:len(waits) - limit]
                    keep = waits[len(waits) - limit:]
                    for k, w in enumerate(extra):
                        ev = mybir.InstEventSemaphore(
                            name=f"{ins.name}-xw{k}", ins=[], outs=[])
                        ev.engine = ins.engine
                        ev.sync_info = mybir.SyncInfo(on_wait=[w], on_update=[])
                        new.append(ev)
                        n_split += 1
                    si.on_wait = keep
                    changed = True
                new.append(ins)
            if changed:
                blk.instructions = new
    return n_split


# ---------------------------------------------------------------------------
# Config
# ---------------------------------------------------------------------------

def make_cfg(n=30000, f_in=4096, hid=128, heads=4, cls=6, cores=8):
    assert hid == 128
    npc = n // cores
    assert npc * cores == n
    blocks = (npc + P - 1) // P
    hc = heads * hid
    cfg = dict(n=n, f_in=f_in, hid=hid, heads=heads, cls=cls, cores=cores,
               npc=npc, blocks=blocks, hc=hc,
               kt=f_in // P,
               zc=hc + 2 * heads,          # [h | asrc | adst]
               z2c=16)                      # [z2(cls) | asrc2 | adst2 | pad]
    assert f_in % P == 0
    return cfg


# ---------------------------------------------------------------------------
# Host-side edge preprocessing
# ---------------------------------------------------------------------------

def preprocess_edges(edge_index, cfg):
    n, cores, npc, blocks = cfg["n"], cfg["cores"], cfg["npc"], cfg["blocks"]
    src = np.concatenate([edge_index[0].astype(np.int64), np.arange(n, dtype=np.int64)])
    dst = np.concatenate([edge_index[1].astype(np.int64), np.arange(n, dtype=np.int64)])
    order = np.argsort(dst, kind="stable")
    src_s, dst_s = src[order], dst[order]

    lists = [[None] * blocks for _ in range(cores)]
    counts = np.zeros((cores, blocks), dtype=np.int64)
    for c in range(cores):
        lo = np.searchsorted(dst_s, c * npc)
        hi = np.searchsorted(dst_s, (c + 1) * npc)
        dloc = dst_s[lo:hi] - c * npc
        sseg = src_s[lo:hi]
        blk = dloc // P
        for b in range(blocks):
            m = blk == b
            lists[c][b] = (sseg[m], (dloc[m] - b * P).astype(np.float32))
            counts[c, b] = int(m.sum())

    ncb = [int(np.ceil(counts[:, b].max() / P)) for b in range(blocks)]
    c1 = int(sum(ncb))
    srcg = np.zeros((cores, P, c1), np.int32)
    slot = np.full((cores, P, c1), 1000.0, np.float32)
    for c in range(cores):
        ci = 0
        for b in range(blocks):
            s_arr, sl_arr = lists[c][b]
            ne = len(s_arr)
            cap = ncb[b] * P
            buf_s = np.zeros(cap, np.int64)
            buf_s[:ne] = s_arr
            buf_sl = np.full(cap, 1000.0, np.float32)
            buf_sl[:ne] = sl_arr
            srcg[c][:, ci:ci + ncb[b]] = buf_s.reshape(ncb[b], P).T
            slot[c][:, ci:ci + ncb[b]] = buf_sl.reshape(ncb[b], P).T
            ci += ncb[b]
    return ncb, srcg, slot


def host_weights(inputs, cfg):
    heads, hid, hc, cls = cfg["heads"], cfg["hid"], cfg["hc"], cfg["cls"]
    W1, a_s1, a_d1 = inputs["W1"], inputs["a_src1"], inputs["a_dst1"]
    W2, a_s2, a_d2 = inputs["W2"], inputs["a_src2"], inputs["a_dst2"]
    A1s = np.zeros((hc, heads), np.float32)
    A1d = np.zeros((hc, heads), np.float32)
    for h in range(heads):
        A1s[h * hid:(h + 1) * hid, h] = a_s1[h]
        A1d[h * hid:(h + 1) * hid, h] = a_d1[h]
    w1a = np.concatenate([W1 @ A1s, W1 @ A1d], axis=1).astype(np.float32)  # [f_in, 2*heads]
    w2e = np.zeros((hc, cfg["z2c"]), np.float32)
    w2e[:, :cls] = W2
    w2e[:, cls:cls + 1] = W2 @ a_s2.T
    w2e[:, cls + 1:cls + 2] = W2 @ a_d2.T
    return w1a, w2e


# ---------------------------------------------------------------------------
# Device program
# ---------------------------------------------------------------------------

def build_program(cfg, ncb, split_waits=True, debug_out=False, phases="full"):
    n, f_in, heads, hid, cls = cfg["n"], cfg["f_in"], cfg["heads"], cfg["hid"], cfg["cls"]
    cores, npc, blocks = cfg["cores"], cfg["npc"], cfg["blocks"]
    kt, hc, zc, z2c = cfg["kt"], cfg["hc"], cfg["zc"], cfg["z2c"]
    c1 = sum(ncb)
    na = 2 * heads  # alpha cols in z
    rg = [list(range(cores))]

    nc = bass.Bass("TRN2", target_bir_lowering=False, debug=False)

    xt = nc.dram_tensor("xt", [f_in, npc], F32, kind="ExternalInput")
    w1h = nc.dram_tensor("w1h", [f_in, hc], F32, kind="ExternalInput")
    w1a = nc.dram_tensor("w1a", [f_in, na], F32, kind="ExternalInput")
    w2e = nc.dram_tensor("w2e", [hc, z2c], F32, kind="ExternalInput")
    b1 = nc.dram_tensor("b1", [1, hc], F32, kind="ExternalInput")
    b2 = nc.dram_tensor("b2", [1, cls], F32, kind="ExternalInput")
    iota_d = nc.dram_tensor("iota", [P, P], F32, kind="ExternalInput")
    ident_d = nc.dram_tensor("ident", [P, P], F32, kind="ExternalInput")
    srcg_d = nc.dram_tensor("srcg", [P, c1], I32, kind="ExternalInput")
    slot_d = nc.dram_tensor("slot", [P, c1], F32, kind="ExternalInput")

    z_loc = nc.dram_tensor("z_loc", [npc, zc], F32)
    z_full = nc.dram_tensor("z_full", [n, zc], F32, addr_space="Shared")
    z2_loc = nc.dram_tensor("z2_loc", [npc, z2c], F32)
    z2_full = nc.dram_tensor("z2_full", [n, z2c], F32, addr_space="Shared")
    out_loc = nc.dram_tensor("out_loc", [npc, cls], F32, kind="ExternalOutput")
    if debug_out:
        z_dbg = nc.dram_tensor("z_dbg", [npc, zc], F32, kind="ExternalOutput")
        h2_dbg = nc.dram_tensor("h2_dbg", [npc, hc], F32, kind="ExternalOutput")
        z2_dbg = nc.dram_tensor("z2_dbg", [npc, z2c], F32, kind="ExternalOutput")

    with tile.TileContext(nc) as tc:
        with tc.tile_pool(name="const", bufs=1) as cpool, \
             tc.tile_pool(name="work", bufs=6) as wpool, \
             tc.tile_pool(name="gath", bufs=10) as gpool, \
             tc.tile_pool(name="xin", bufs=2) as xpool, \
             tc.tile_pool(name="elu", bufs=2) as epool, \
             tc.tile_pool(name="pbig", bufs=2, space="PSUM") as pbig, \
             tc.tile_pool(name="pa4", bufs=1, space="PSUM") as pa4, \
             tc.tile_pool(name="ppt", bufs=3, space="PSUM") as ppt, \
             tc.tile_pool(name="psmz", bufs=2, space="PSUM") as psmz:

            # ---- constants / metadata ----
            w1h_sb = cpool.tile([P, kt * hc], F32)
            for k in range(kt):
                nc.sync.dma_start(out=w1h_sb[:, k * hc:(k + 1) * hc],
                                  in_=w1h[k * P:(k + 1) * P, :])
            w1a_sb = cpool.tile([P, kt * na], F32)
            for k in range(kt):
                nc.sync.dma_start(out=w1a_sb[:, k * na:(k + 1) * na],
                                  in_=w1a[k * P:(k + 1) * P, :])
            w2e_sb = cpool.tile([P, (hc // P) * z2c], F32)
            for q in range(hc // P):
                nc.sync.dma_start(out=w2e_sb[:, q * z2c:(q + 1) * z2c],
                                  in_=w2e[q * P:(q + 1) * P, :])
            b1b = cpool.tile([P, hc], F32)
            nc.sync.dma_start(out=b1b[:], in_=b1[:].to_broadcast((P, hc)))
            b2b = cpool.tile([P, cls], F32)
            nc.sync.dma_start(out=b2b[:], in_=b2[:].to_broadcast((P, cls)))
            iota_sb = cpool.tile([P, P], F32)
            nc.sync.dma_start(out=iota_sb[:], in_=iota_d[:])
            ident_sb = cpool.tile([P, P], F32)
            nc.sync.dma_start(out=ident_sb[:], in_=ident_d[:])
            srcg_sb = cpool.tile([P, c1], I32)
            nc.sync.dma_start(out=srcg_sb[:], in_=srcg_d[:])
            slot_sb = cpool.tile([P, c1], F32)
            nc.sync.dma_start(out=slot_sb[:], in_=slot_d[:])

            adst_all = cpool.tile([P, blocks * heads], F32)
            nc.vector.memset(adst_all[:], 0.0)
            adst2_all = cpool.tile([P, blocks], F32)
            nc.vector.memset(adst2_all[:], 0.0)
            z2_all = cpool.tile([P, blocks * z2c], F32)
            nc.vector.memset(z2_all[:], 0.0)

            # ---- Phase B: z = x @ [W1h | W1a] for local nodes ----
            xt_view = xt[:].rearrange("(t p) n -> p t n", p=P)
            for m in range(blocks):
                rows = min(P, npc - m * P)
                xtm = xpool.tile([P, kt, P], F32, tag="xt")
                nc.sync.dma_start(out=xtm[:, :, :rows],
                                  in_=xt_view[:, :, m * P:m * P + rows])
                ph = pbig.tile([P, hc], F32, tag="big")
                pa = pa4.tile([P, 32], F32, tag="a4")
                for k in range(kt):
                    nc.tensor.matmul(ph[:rows, :], lhsT=xtm[:, k, :rows],
                                     rhs=w1h_sb[:, k * hc:(k + 1) * hc],
                                     start=(k == 0), stop=(k == kt - 1))
                    nc.tensor.matmul(pa[:rows, :na], lhsT=xtm[:, k, :rows],
                                     rhs=w1a_sb[:, k * na:(k + 1) * na],
                                     start=(k == 0), stop=(k == kt - 1))
                z_t = wpool.tile([P, zc], F32, tag="zt")
                nc.vector.tensor_copy(out=z_t[:rows, 0:hc], in_=ph[:rows, :])
                nc.scalar.copy(out=z_t[:rows, hc:hc + na], in_=pa[:rows, 0:na])
                nc.vector.tensor_copy(out=adst_all[:rows, m * heads:(m + 1) * heads],
                                      in_=pa[:rows, heads:2 * heads])
                nc.sync.dma_start(out=z_loc[m * P:m * P + rows, :], in_=z_t[:rows, :])
                if debug_out:
                    nc.sync.dma_start(out=z_dbg[m * P:m * P + rows, :], in_=z_t[:rows, :])

            # ---- Phase C: AllGather z ----
            nc.gpsimd.collective_compute(
                "AllGather", ALU.bypass, ins=[z_loc[:]], outs=[z_full[:]],
                replica_groups=rg)

            if phases == "B":
                dummy = wpool.tile([P, cls], F32, tag="dummy")
                nc.vector.memset(dummy[:], 0.0)
                for b in range(blocks):
                    rows = min(P, npc - b * P)
                    nc.sync.dma_start(out=out_loc[b * P:b * P + rows, :],
                                      in_=dummy[:rows, :])
            # ---- Phase D: layer-1 edge pass + layer-2 prep ----
            ci = 0
            for b in (range(blocks) if phases != "B" else []):
                rows = min(P, npc - b * P)
                pout = pbig.tile([P, hc], F32, tag="big")
                pden = pa4.tile([P, 32], F32, tag="a4")
                for j in range(ncb[b]):
                    zr = gpool.tile([P, zc], F32, tag="zr")
                    nc.gpsimd.indirect_dma_start(
                        out=zr[:], out_offset=None, in_=z_full[:, :],
                        in_offset=bass.IndirectOffsetOnAxis(
                            ap=srcg_sb[:, ci:ci + 1], axis=0))
                    s_es = wpool.tile([P, P], F32, tag="ses")
                    nc.vector.tensor_tensor(
                        out=s_es[:], in0=slot_sb[:, ci:ci + 1].to_broadcast((P, P)),
                        in1=iota_sb[:], op=ALU.is_equal)
                    pt = ppt.tile([P, P], F32, tag="pt")
                    nc.tensor.transpose(out=pt[:], in_=s_es[:], identity=ident_sb[:])
                    s_se = wpool.tile([P, P], F32, tag="sse")
                    nc.vector.tensor_copy(out=s_se[:], in_=pt[:])
                    pe = ppt.tile([P, P], F32, tag="pt")
                    nc.tensor.matmul(pe[:, 0:heads], lhsT=s_se[:],
                                     rhs=adst_all[:, b * heads:(b + 1) * heads],
                                     start=True, stop=True)
                    tl = wpool.tile([P, heads], F32, tag="tl")
                    nc.vector.tensor_tensor(out=tl[:], in0=pe[:, 0:heads],
                                            in1=zr[:, hc:hc + heads], op=ALU.add)
                    lr = wpool.tile([P, heads], F32, tag="lr")
                    nc.vector.scalar_tensor_tensor(out=lr[:], in0=tl[:], scalar=NEG_SLOPE,
                                                   in1=tl[:], op0=ALU.mult, op1=ALU.max)
                    pv = wpool.tile([P, heads], F32, tag="pv")
                    nc.scalar.activation(pv[:], lr[:], AF.Exp)
                    # scale the 4 head column-blocks of zr by p (spread engines)
                    for h in range(heads):
                        sl = slice(h * hid, (h + 1) * hid)
                        if h % 2 == 0:
                            nc.vector.tensor_scalar_mul(zr[:, sl], zr[:, sl],
                                                        pv[:, h:h + 1])
                        else:
                            nc.scalar.activation(zr[:, sl], zr[:, sl], AF.Copy,
                                                 scale=pv[:, h:h + 1])
                    nc.tensor.matmul(pout[:, :], lhsT=s_es[:], rhs=zr[:, 0:hc],
                                     start=(j == 0), stop=(j == ncb[b] - 1))
                    nc.tensor.matmul(pden[:, 0:heads], lhsT=s_es[:], rhs=pv[:],
                                     start=(j == 0), stop=(j == ncb[b] - 1))
                    ci += 1
                # normalize + bias + ELU -> h2; z2 = h2 @ w2e
                rec = wpool.tile([P, heads], F32, tag="rec")
                nc.vector.reciprocal(rec[:], pden[:, 0:heads])
                u = epool.tile([P, hc], F32, tag="u")
                for h in range(heads):
                    sl = slice(h * hid, (h + 1) * hid)
                    nc.vector.scalar_tensor_tensor(
                        out=u[:, sl], in0=pout[:, sl], scalar=rec[:, h:h + 1],
                        in1=b1b[:, sl], op0=ALU.mult, op1=ALU.add)
                mneg = epool.tile([P, hc], F32, tag="mneg")
                nc.vector.tensor_scalar_min(mneg[:], u[:], 0.0)
                rpos = epool.tile([P, hc], F32, tag="rpos")
                nc.vector.scalar_tensor_tensor(out=rpos[:], in0=mneg[:], scalar=-1.0,
                                               in1=u[:], op0=ALU.mult, op1=ALU.add)
                ex = epool.tile([P, hc], F32, tag="ex")
                nc.scalar.activation(ex[:], mneg[:], AF.Exp)
                h2t = epool.tile([P, hc], F32, tag="h2t")
                nc.vector.scalar_tensor_tensor(out=h2t[:], in0=ex[:], scalar=-1.0,
                                               in1=rpos[:], op0=ALU.add, op1=ALU.add)
                pz2 = psmz.tile([P, z2c], F32, tag="smz")
                for q in range(hc // P):
                    ptq = ppt.tile([P, P], F32, tag="pt")
                    nc.tensor.transpose(out=ptq[:], in_=h2t[:, q * P:(q + 1) * P],
                                        identity=ident_sb[:])
                    h2tq = wpool.tile([P, P], F32, tag="h2q")
                    nc.vector.tensor_copy(out=h2tq[:], in_=ptq[:])
                    nc.tensor.matmul(pz2[:, :], lhsT=h2tq[:],
                                     rhs=w2e_sb[:, q * z2c:(q + 1) * z2c],
                                     start=(q == 0), stop=(q == hc // P - 1))
                nc.scalar.copy(out=z2_all[:rows, b * z2c:(b + 1) * z2c],
                               in_=pz2[:rows, :])
                nc.vector.tensor_copy(out=adst2_all[:rows, b:b + 1],
                                      in_=pz2[:rows, cls + 1:cls + 2])
                nc.sync.dma_start(out=z2_loc[b * P:b * P + rows, :],
                                  in_=z2_all[:rows, b * z2c:(b + 1) * z2c])
                if debug_out:
                    nc.sync.dma_start(out=h2_dbg[b * P:b * P + rows, :], in_=h2t[:rows, :])
                    nc.sync.dma_start(out=z2_dbg[b * P:b * P + rows, :],
                                      in_=z2_all[:rows, b * z2c:(b + 1) * z2c])

            # ---- Phase E: AllGather z2 ----
            if phases not in ("B",):
                nc.gpsimd.collective_compute(
                    "AllGather", ALU.bypass, ins=[z2_loc[:]], outs=[z2_full[:]],
                    replica_groups=rg)
            if phases == "BD":
                dummy = wpool.tile([P, cls], F32, tag="dummy")
                nc.vector.memset(dummy[:], 0.0)
                for b in range(blocks):
                    rows = min(P, npc - b * P)
                    nc.sync.dma_start(out=out_loc[b * P:b * P + rows, :],
                                      in_=dummy[:rows, :])

            # ---- Phase F: layer-2 edge pass ----
            ci = 0
            for b in (range(blocks) if phases == "full" else []):
                rows = min(P, npc - b * P)
                p2s = psmz.tile([P, z2c], F32, tag="smz")
                for j in range(ncb[b]):
                    z2r = gpool.tile([P, z2c], F32, tag="z2r")
                    nc.gpsimd.indirect_dma_start(
                        out=z2r[:], out_offset=None, in_=z2_full[:, :],
                        in_offset=bass.IndirectOffsetOnAxis(
                            ap=srcg_sb[:, ci:ci + 1], axis=0))
                    s_es = wpool.tile([P, P], F32, tag="ses")
                    nc.vector.tensor_tensor(
                        out=s_es[:], in0=slot_sb[:, ci:ci + 1].to_broadcast((P, P)),
                        in1=iota_sb[:], op=ALU.is_equal)
                    pt = ppt.tile([P, P], F32, tag="pt")
                    nc.tensor.transpose(out=pt[:], in_=s_es[:], identity=ident_sb[:])
                    s_se = wpool.tile([P, P], F32, tag="sse")
                    nc.vector.tensor_copy(out=s_se[:], in_=pt[:])
                    pe2 = ppt.tile([P, P], F32, tag="pt")
                    nc.tensor.matmul(pe2[:, 0:1], lhsT=s_se[:],
                                     rhs=adst2_all[:, b:b + 1], start=True, stop=True)
                    tl2 = wpool.tile([P, 1], F32, tag="tl2")
                    nc.vector.tensor_tensor(out=tl2[:], in0=pe2[:, 0:1],
                                            in1=z2r[:, cls:cls + 1], op=ALU.add)
                    lr2 = wpool.tile([P, 1], F32, tag="lr2")
                    nc.vector.scalar_tensor_tensor(out=lr2[:], in0=tl2[:], scalar=NEG_SLOPE,
                                                   in1=tl2[:], op0=ALU.mult, op1=ALU.max)
                    p2v = wpool.tile([P, 1], F32, tag="p2v")
                    nc.scalar.activation(p2v[:], lr2[:], AF.Exp)
                    m2p = wpool.tile([P, 8], F32, tag="m2p")
                    nc.vector.tensor_scalar_mul(m2p[:, 0:cls], z2r[:, 0:cls],
                                                p2v[:, 0:1])
                    nc.vector.tensor_copy(out=m2p[:, cls:cls + 1], in_=p2v[:])
                    nc.tensor.matmul(p2s[:, 0:cls + 1], lhsT=s_es[:],
                                     rhs=m2p[:, 0:cls + 1],
                                     start=(j == 0), stop=(j == ncb[b] - 1))
                    ci += 1
                r2 = wpool.tile([P, 1], F32, tag="r2")
                nc.vector.reciprocal(r2[:], p2s[:, cls:cls + 1])
                o2 = wpool.tile([P, cls], F32, tag="o2")
                nc.vector.scalar_tensor_tensor(
                    out=o2[:], in0=p2s[:, 0:cls], scalar=r2[:, 0:1],
                    in1=b2b[:], op0=ALU.mult, op1=ALU.add)
                nc.sync.dma_start(out=out_loc[b * P:b * P + rows, :],
                                  in_=o2[:rows, :])

    if split_waits:
        split_sync_waits(nc)
    return nc


# ---------------------------------------------------------------------------
# Host orchestration
# ---------------------------------------------------------------------------

def make_in_maps(inputs, cfg, ncb, srcg, slot):
    n, f_in, cores, npc, hc = cfg["n"], cfg["f_in"], cfg["cores"], cfg["npc"], cfg["hc"]
    x = np.asarray(inputs["x"], np.float32)
    xT = np.ascontiguousarray(x.T)
    w1a, w2e = host_weights(inputs, cfg)
    w1h = np.ascontiguousarray(np.asarray(inputs["W1"], np.float32))
    b1 = np.asarray(inputs["b1"], np.float32).reshape(1, hc)
    b2 = np.asarray(inputs["b2"], np.float32).reshape(1, cfg["cls"])
    iota = np.tile(np.arange(P, dtype=np.float32), (P, 1))
    ident = np.eye(P, dtype=np.float32)
    in_maps = []
    for c in range(cores):
        in_maps.append({
            "xt": np.ascontiguousarray(xT[:, c * npc:(c + 1) * npc]),
            "w1h": w1h, "w1a": w1a, "w2e": w2e,
            "b1": b1, "b2": b2, "iota": iota, "ident": ident,
            "srcg": srcg[c], "slot": slot[c],
        })
    return in_maps


_cache = {}


def _get_program(cfg_key, cfg, ncb):
    if cfg_key not in _cache:
        _cache[cfg_key] = build_program(cfg, ncb)
    return _cache[cfg_key]


def kernel(**inputs):
    cfg = make_cfg()
    edge_index = np.asarray(inputs["edge_index"])
    ncb, srcg, slot = preprocess_edges(edge_index, cfg)
    in_maps = make_in_maps(inputs, cfg, ncb, srcg, slot)
    cfg_key = ("full", tuple(ncb))
    nc = _get_program(cfg_key, cfg, ncb)

    from concourse import bass2jax
    results = bass2jax.run_bass_via_pjrt(nc, in_maps, n_cores=cfg["cores"])
    out = np.concatenate([r["out_loc"] for r in results], axis=0)
    return out.astype(np.float32)

